# revision 17
# baseline (speedup 1.0000x reference)
"""Trainium2 Bass kernel for MatchingLayerL2:
   out = log_softmax(-sqrt(||x_i - y_j||^2) / std_j, axis=1)

x: [4096, 128] f32, y: [32768, 128] f32, std: [32768] f32 -> out [4096, 32768] f32.

Strategy: shard rows of x across 8 cores (512 rows each); y/std replicated.
Per core:
  rstd2_j = 1/std_j^2
  q_ij = rstd2_j * dist2_ij = (-2 x_i) . (y_j rstd2_j) + a_i rstd2_j + (b_j rstd2_j)
       (a = ||x||^2 rowwise, b = ||y hat||^2 * std^2 rowwise)
  s_ij = sqrt(q_ij) = dist_ij * rstd_j          (fp16 in SBUF)
  out_ij = -s_ij - ln(sum_j exp(-s_ij))          (no max-shift: s in [7,47])
Main matmul in bf16 (K=128); the rank-2 correction a*r + b*r is added with a
K=5 bf16 matmul whose rows are hi/lo bf16 splits for fp32-grade accuracy.
The 5 correction rows are staged through an internal DRAM tensor because a
[5, M] SBUF tile would charge M*2 bytes across all 128 partitions.
"""

import os
import sys

sys.path.insert(0, "/root/.axon_site/_ro/trn_rl_repo")

import numpy as np
from contextlib import ExitStack

import concourse.bass as bass
from concourse import bacc
import concourse.tile as tile
from concourse.tile import add_dep_helper
from concourse import mybir, masks
from concourse.bass_utils import run_bass_kernel_spmd

F32 = mybir.dt.float32
BF16 = mybir.dt.bfloat16
FP16 = mybir.dt.float16
AF = mybir.ActivationFunctionType
ALU = mybir.AluOpType
AX = mybir.AxisListType

N_CORES = 8
D = 128
P = 128


def build_nc(rows, M, final_sub_engine="vector"):
    """Build the Bass module for one core: x shard [rows, D], y [M, D], std [M]."""
    NB = rows // P          # row blocks of 128
    NCH = M // 512          # y chunks (512 y-rows each)
    NS = M // 2048          # s tiles per block
    nA = M // P             # layout-A columns: v[q, t] = v[t*128 + q]

    nc = bacc.Bacc("TRN2", target_bir_lowering=False, debug=False, num_swdge_queues=4)
    x_d = nc.declare_dram_parameter("x", [rows, D], F32, isOutput=False)
    y_d = nc.declare_dram_parameter("y", [M, D], F32, isOutput=False)
    std_d = nc.declare_dram_parameter("std", [M], F32, isOutput=False)
    out_d = nc.declare_dram_parameter("out", [rows, M], F32, isOutput=True)
    corr_d = nc.dram_tensor("corr", [5, M], BF16, kind="Internal")

    act_prev = [None]

    def act(*a, **k):
        inst = nc.scalar.activation(*a, **k)
        if act_prev[0] is not None:
            add_dep_helper(inst.ins, act_prev[0].ins, sync=False, reason="act order")
        act_prev[0] = inst
        return inst

    with tile.TileContext(nc) as tc, ExitStack() as ctx:
        pool = lambda name, bufs, space="SBUF": ctx.enter_context(
            tc.tile_pool(name=name, bufs=bufs, space=space)
        )

        const_p = pool("const", 1)
        ystage_p = pool("ystage", 2)
        ybar_p = pool("ybar", 2)
        yT_p = pool("yT", NCH)
        sqn_p = pool("sqn", 2)
        colsA_p = pool("colsA", 1)      # stdA, rstdA, rA, std2A  (f32 [128, nA])
        colsAh_p = pool("colsAh", 1)    # r hi/lo bf16 [128, nA]
        bcols_p = pool("bcols", 1)      # b2A f32 [128, nA]
        bg_p = pool("bg", 2)            # per-group bhat tiles [128, 32]
        rowT_p = pool("rowT", 2)        # transposed row chunks [*, 128] bf16
        xa_p = pool("xa", 1)
        acol_p = pool("acol", 1)
        lhs_p = pool("lhs", 1)
        lhsc_p = pool("lhsc", NB)
        corrt_p = pool("corrt", 2)
        s_p = pool("s", NS)
        part_p = pool("part", 2)
        scal_p = pool("scal", 6)
        escr_p = pool("escr", 1)
        ostage_p = pool("ostage", 4)

        mm_ps = pool("mmps", 3, space="PSUM")    # 3 x [128,1024] f32 = 6 banks
        tp_ps = pool("tpps", 2, space="PSUM")    # 2 x [128,512] bf16 = 2 banks

        # ---------------- constants ----------------
        ident = const_p.tile([P, P], BF16)
        masks.make_identity(nc, ident[:])
        identf = const_p.tile([P, P], F32)
        masks.make_identity(nc, identf[:])

        # ---------------- std-derived quantities (layout A) ----------------
        # stdA[q, t] = std[128 t + q]: load natural [t, q] tiles, PE-transpose.
        stdA = colsA_p.tile([P, nA], F32)
        for c in range((nA + P - 1) // P):
            h = min(P, nA - c * P)
            stn = rowT_p.tile([P, P], F32, tag="stn")
            nc.sync.dma_start(
                out=stn[0:h, :],
                in_=std_d[P * P * c : P * (P * c + h)].rearrange(
                    "(t q) -> t q", q=P
                ),
            )
            tpf = tp_ps.tile([P, P], F32, tag="tp")
            nc.tensor.transpose(tpf[:, 0:h], stn[0:h, :], identf[:])
            nc.vector.tensor_copy(stdA[:, c * P : c * P + h], tpf[:, 0:h])
        rstdA = colsA_p.tile([P, nA], F32)
        nc.vector.reciprocal(rstdA[:], stdA[:])
        rA = colsA_p.tile([P, nA], F32)
        nc.vector.tensor_tensor(rA[:], rstdA[:], rstdA[:], op=ALU.mult)
        std2A = colsA_p.tile([P, nA], F32)
        nc.vector.tensor_tensor(std2A[:], stdA[:], stdA[:], op=ALU.mult)
        rhiA = colsAh_p.tile([P, nA], BF16)
        nc.vector.tensor_copy(rhiA[:], rA[:])
        rloA = colsAh_p.tile([P, nA], BF16)
        nc.vector.tensor_tensor(rloA[:], rA[:], rhiA[:], op=ALU.subtract)
        # corr rows 0,1 = r_hi (pairs with a_hi, a_lo), row 2 = r_lo (pairs a_hi).
        # Transpose [128, 128]-blocks to row-major before storing (fast DMA).
        for row, src in ((0, rhiA), (1, rhiA), (2, rloA)):
            for c in range((nA + P - 1) // P):
                w = min(P, nA - c * P)
                tp = tp_ps.tile([P, 512], BF16, tag="tp")
                nc.tensor.transpose(
                    tp[0:w, 0:P], src[:, c * P : c * P + w], ident[:]
                )
                rt = rowT_p.tile([P, P], BF16, tag="rowT")
                nc.vector.tensor_copy(rt[0:w, :], tp[0:w, 0:P])
                nc.gpsimd.dma_start(
                    out=corr_d[row, c * P * P : (c * P + w) * P].rearrange(
                        "(t q) -> t q", q=P
                    ),
                    in_=rt[0:w, :],
                )

        # ---------------- x side: lhsT_main = (-2x)^T bf16, a = ||x||^2 ----------------
        xstage = xa_p.tile([P, NB, D], F32)
        nc.sync.dma_start(
            out=xstage[:], in_=x_d[:, :].rearrange("(c p) d -> p c d", p=P)
        )
        xsq = xa_p.tile([P, NB, D], F32)
        nc.vector.tensor_tensor(xsq[:], xstage[:], xstage[:], op=ALU.mult)
        a_cols = acol_p.tile([P, NB], F32)
        nc.vector.tensor_reduce(a_cols[:], xsq[:], axis=AX.X, op=ALU.add)
        ahi_col = acol_p.tile([P, NB], BF16)
        nc.vector.tensor_copy(ahi_col[:], a_cols[:])
        alo_col = acol_p.tile([P, NB], BF16)
        nc.vector.tensor_tensor(alo_col[:], a_cols[:], ahi_col[:], op=ALU.subtract)

        lhsT_main = lhs_p.tile([P, rows], BF16)
        xbar = xa_p.tile([P, NB, D], BF16, tag="xbar")
        nc.vector.tensor_scalar(xbar[:], xstage[:], -2.0, None, op0=ALU.mult)
        for c in range(NB):
            tp = tp_ps.tile([P, 512], BF16, tag="tp")
            nc.tensor.transpose(tp[:, 0:P], xbar[:, c, :], ident[:])
            nc.vector.tensor_copy(lhsT_main[:, c * P : (c + 1) * P], tp[:, 0:P])

        # lhsT_corr per block: rows [a_hi; a_lo; a_hi; 1; 1] as [5, 128] bf16
        lhsT_corr = []
        for b in range(NB):
            asm = acol_p.tile([P, 8], BF16, tag="asm")
            nc.vector.tensor_copy(asm[:, 0:1], ahi_col[:, b : b + 1])
            nc.vector.tensor_copy(asm[:, 1:2], alo_col[:, b : b + 1])
            nc.vector.tensor_copy(asm[:, 2:3], ahi_col[:, b : b + 1])
            nc.vector.memset(asm[:, 3:5], 1.0)
            tp = tp_ps.tile([P, 512], BF16, tag="tp")
            nc.tensor.transpose(tp[0:5, 0:P], asm[:, 0:5], ident[:])
            lc = lhsc_p.tile([5, P], BF16)
            nc.vector.tensor_copy(lc[:], tp[0:5, 0:P])
            lhsT_corr.append(lc)

        # ---------------- y prologue: yT tiles + b-hat rows ----------------
        b2A = bcols_p.tile([P, nA], F32)
        yT = []
        for t in range(NCH):
            yst = ystage_p.tile([P, 4, D], F32)
            nc.sync.dma_start(
                out=yst[:],
                in_=y_d[512 * t : 512 * (t + 1), :].rearrange(
                    "(c p) d -> p c d", p=P
                ),
            )
            yb = ybar_p.tile([P, 4, D], BF16)
            for c in range(4):
                nc.vector.tensor_scalar(
                    yb[:, c, :],
                    yst[:, c, :],
                    rA[:, 4 * t + c : 4 * t + c + 1],
                    None,
                    op0=ALU.mult,
                )
            # b2 = sum_d yhat^2 (layout A cols), from the bf16 scaled tiles
            sqn = sqn_p.tile([P, 4, D], BF16)
            nc.vector.tensor_tensor(sqn[:], yb[:], yb[:], op=ALU.mult)
            nc.vector.tensor_reduce(
                b2A[:, 4 * t : 4 * t + 4], sqn[:], axis=AX.X, op=ALU.add
            )
            tp = tp_ps.tile([P, 512], BF16, tag="tp")
            for c in range(4):
                nc.tensor.transpose(tp[:, c * P : (c + 1) * P], yb[:, c, :], ident[:])
            yt = yT_p.tile([P, 512], BF16)
            nc.vector.tensor_copy(yt[:], tp[:])
            yT.append(yt)
            # after each group of 4 chunks (2048 j's), build b-hat rows -> DRAM
            if t % 4 == 3:
                g0 = 4 * (t - 3)
                csl = slice(g0, g0 + 16)
                bhat = bg_p.tile([P, 16], F32, tag="bhat")
                nc.vector.tensor_tensor(bhat[:], b2A[:, csl], std2A[:, csl], op=ALU.mult)
                bhi = bg_p.tile([P, 16], BF16, tag="bhi")
                nc.vector.tensor_copy(bhi[:], bhat[:])
                blo = bg_p.tile([P, 16], BF16, tag="blo")
                nc.vector.tensor_tensor(blo[:], bhat[:], bhi[:], op=ALU.subtract)
                for row, src in ((3, bhi), (4, blo)):
                    tp2 = tp_ps.tile([P, 512], BF16, tag="tp")
                    nc.tensor.transpose(tp2[0:16, 0:P], src[:], ident[:])
                    rt = rowT_p.tile([P, P], BF16, tag="rowT")
                    nc.vector.tensor_copy(rt[0:16, :], tp2[0:16, 0:P])
                    nc.gpsimd.dma_start(
                        out=corr_d[row, P * g0 : P * (g0 + 16)].rearrange(
                            "(t q) -> t q", q=P
                        ),
                        in_=rt[0:16, :],
                    )

        # ---------------- main loop over row blocks ----------------
        fsub = nc.gpsimd if final_sub_engine == "gpsimd" else nc.vector
        for b in range(NB):
            partials = part_p.tile([P, NS], F32)
            # phase 1: all sqrts of the block (batched per ACT table set)
            s_tiles = []
            for st in range(NS):
                s_t = s_p.tile([P, 2048], FP16)
                for h in range(2):
                    jg = 2 * st + h
                    if jg % 4 == 0:
                        ct = corrt_p.tile([5, 4096], BF16)
                        nc.gpsimd.dma_start(
                            out=ct[:], in_=corr_d[:, 1024 * jg : 1024 * (jg + 4)]
                        )
                    co = 1024 * (jg % 4)
                    mm = mm_ps.tile([P, 1024], F32)
                    # mains first, then corrs: one lhsT switch per psum tile
                    for q in range(2):
                        nc.tensor.matmul(
                            mm[:, 512 * q : 512 * (q + 1)],
                            lhsT_main[:, b * P : (b + 1) * P],
                            yT[2 * jg + q][:],
                            start=True,
                            stop=False,
                        )
                    for q in range(2):
                        nc.tensor.matmul(
                            mm[:, 512 * q : 512 * (q + 1)],
                            lhsT_corr[b][:],
                            ct[:, co + 512 * q : co + 512 * (q + 1)],
                            start=False,
                            stop=True,
                        )
                    act(s_t[:, 1024 * h : 1024 * (h + 1)], mm[:], AF.Sqrt)
                s_tiles.append(s_t)
            # phase 2: all exps (single exp-table load per block)
            for st in range(NS):
                es = escr_p.tile([P, 2048], BF16)
                act(
                    es[:],
                    s_tiles[st][:],
                    AF.Exp,
                    scale=-1.0,
                    accum_out=partials[:, st : st + 1],
                )
            S = scal_p.tile([P, 1], F32)
            nc.vector.tensor_reduce(S[:], partials[:], axis=AX.X, op=ALU.add)
            lnS = scal_p.tile([P, 1], F32)
            act(lnS[:], S[:], AF.Ln)
            negc = scal_p.tile([P, 1], F32)
            nc.vector.tensor_scalar(negc[:], lnS[:], -1.0, None, op0=ALU.mult)
            for st in range(NS):
                for h in range(2):
                    og = ostage_p.tile([P, 1024], F32)
                    fsub.tensor_scalar(
                        og[:],
                        s_tiles[st][:, 1024 * h : 1024 * (h + 1)],
                        -1.0,
                        negc[:],
                        op0=ALU.mult,
                        op1=ALU.add,
                    )
                    j0 = 2048 * st + 1024 * h
                    nc.sync.dma_start(
                        out=out_d[b * P : (b + 1) * P, j0 : j0 + 1024],
                        in_=og[:],
                    )

    nc.finalize()
    return nc


_NC_CACHE = {}


def _get_nc(rows, M):
    key = (rows, M)
    if key not in _NC_CACHE:
        _NC_CACHE[key] = build_nc(rows, M)
    return _NC_CACHE[key]


def kernel(x: np.ndarray, y: np.ndarray, std: np.ndarray) -> np.ndarray:
    x = np.ascontiguousarray(x, dtype=np.float32)
    y = np.ascontiguousarray(y, dtype=np.float32)
    std = np.ascontiguousarray(std, dtype=np.float32)
    N, M = x.shape[0], y.shape[0]
    rows = N // N_CORES
    nc = _get_nc(rows, M)
    in_maps = [
        {"x": x[c * rows : (c + 1) * rows], "y": y, "std": std}
        for c in range(N_CORES)
    ]
    trace = bool(int(os.environ.get("KERNEL_TRACE", "0")))
    res = run_bass_kernel_spmd(
        nc, in_maps, core_ids=list(range(N_CORES)), trace=trace
    )
    global LAST_RESULT
    LAST_RESULT = res
    return np.concatenate(
        [res.results[c]["out"] for c in range(N_CORES)], axis=0
    ).astype(np.float32)


LAST_RESULT = None
